# revision 4
# baseline (speedup 1.0000x reference)
"""DigitCapsule dynamic-routing kernel for 8 Trainium2 NeuronCores.

Key restructuring: u_hat (B,R,D,O) = 188 MB is NEVER materialized.
  s[b,(d,o)]  = sum_{(r,i)} (c[r,d]*W[r,d,o,i]) * u[b,r,i]      (matmul over (r,i))
  b_upd[r,d]  = sum_{i,o} W[r,d,o,i] * G[(r,i),(d,o)],
  G[(r,i),(d,o)] = sum_b u[b,(r,i)] * v[b,(d,o)]                 (matmul over b)

Sharding: route nodes R=1152 are split 144/core across 8 cores.  Softmax
(over d) and the b-logit update are then fully local; the only collective
is one AllReduce of the partial s per routing iteration (3 total).

Perf notes (v2):
  * All PE operands are bf16 (fp32 matmuls cost 4 cycles/row vs bf16's 1);
    PSUM accumulation and the AllReduce payload stay fp32, as does all
    squash / logit math, so the only precision loss is the bf16 input
    rounding (~0.3% relative, vs a 2e-2 gate).
  * The gpsimd (Pool) queue carries ONLY the collectives plus SBUF-only
    elementwise offload (Hred / CW); u_nat+J loads moved to the DVE queue
    and gated past the uT/Wp loads so neither the Pool sequencer nor the
    serialized DMA transfer engine delays the iteration-0 AllReduce.
  * Softmax is batched: one Exp over all 9 t-tiles + one reduce + one
    reciprocal + one broadcast multiply, instead of 9 per-t Act chains.
  * s goes PSUM -> (per-half DVE drain) -> one [128,320] DMA to the
    collective input; sf comes back in the same layout (no rearrange).
The device tracks s_dev = A*s_true (A=1 normally; iteration 0 skips the
softmax entirely, feeding W straight to mm1, so A = 10 there) and corrects
inside squash: v = s_dev * sqrt(T)/(A^2 + T) with T = sum(s_dev^2).
Iterations 0-1 never materialize v at all: mm2 consumes the AllReduce
output directly and the squash scalar g = sqrt(T)/(A^2+T) folds into the
W*G multiply.
"""

import ml_dtypes
import numpy as np

import concourse.bass as bass
import concourse.mybir as mybir
import concourse.tile as tile
from concourse.bass_utils import run_bass_kernel_spmd
from concourse.tile import add_dep_helper

N_CORES = 8
B, R, D, O, I_CH = 256, 1152, 10, 16, 8
RL = R // N_CORES           # 144 route nodes per core
KRI = RL * I_CH             # 1152 = (r,i) contraction length per core
NT = KRI // 128             # 9 partition tiles of (r,i)
DO = D * O                  # 160
NB = B // 128               # 2 batch halves
N_ITER = 3

f32 = mybir.dt.float32
bf16 = mybir.dt.bfloat16
ALU = mybir.AluOpType
AF = mybir.ActivationFunctionType

_ws_ctr = [0]


def _split_excess_waits(nc, max_waits=1):
    """Walrus in this container only lowers one sync-wait per instruction.
    Hoist excess waits onto NOPs inserted before the instruction on the
    same engine (same-order execution => identical semantics)."""
    n_split = 0
    for f in nc.m.functions:
        for bb in f.blocks:
            out = []
            changed = False
            for ins in bb.instructions:
                si = ins.sync_info
                waits = list(si.on_wait) if (si is not None and si.on_wait) else []
                if len(waits) > max_waits:
                    changed = True
                    n_split += 1
                    head, rest = waits[:-max_waits], waits[-max_waits:]
                    while head:
                        chunk, head = head[:max_waits], head[max_waits:]
                        _ws_ctr[0] += 1
                        nop = mybir.InstNoOp(name=f"I-ws{_ws_ctr[0]}")
                        nop.engine = ins.engine
                        nop.sync_info = mybir.SyncInfo(on_wait=chunk, on_update=[])
                        out.append(nop)
                    ins.sync_info = mybir.SyncInfo(
                        on_wait=rest,
                        on_update=list(si.on_update) if si.on_update else [],
                    )
                out.append(ins)
            if changed:
                bb.instructions = out
    return n_split


def _build_nc(reps=1, prewarm=10):
    nc = bass.Bass(
        "TRN2", target_bir_lowering=False, debug=False, num_devices=N_CORES
    )
    un_d = nc.dram_tensor("u_nat", [NB, 128, KRI], bf16, kind="ExternalInput")
    uT_d = nc.dram_tensor("uT", [128, NT, B], bf16, kind="ExternalInput")
    Wp_d = nc.dram_tensor("Wp", [128, NT, DO], bf16, kind="ExternalInput")
    Jm_d = nc.dram_tensor("Jm", [128, 128], f32, kind="ExternalInput")
    v_out_d = nc.dram_tensor("v_out", [NB, 128, DO], f32, kind="ExternalOutput")

    rg = [list(range(N_CORES))]

    with tile.TileContext(nc) as tc:
        with (
            tc.tile_pool(name="persist", bufs=1) as pp_,
            tc.tile_pool(name="iter", bufs=2) as ip_,
            tc.tile_pool(name="small", bufs=2) as sp_,
            tc.tile_pool(name="dram", bufs=2, space="DRAM") as dp_,
            tc.tile_pool(name="ps_s", bufs=2, space="PSUM") as ps_s,
            tc.tile_pool(name="ps_g", bufs=2, space="PSUM") as ps_g,
            tc.tile_pool(name="ps_bd", bufs=2, space="PSUM") as ps_bd,
            tc.tile_pool(name="ps_t", bufs=1, space="PSUM") as ps_t,
        ):
            # ---- persistent tensors ----
            un = pp_.tile([128, NB, KRI], bf16)
            uT = pp_.tile([128, NT, B], bf16)
            Wp = pp_.tile([128, NT, DO], bf16)
            J = pp_.tile([128, 128], f32)
            ones = pp_.tile([128, 128], f32)
            ones16 = pp_.tile([128, 128], bf16)
            blog = pp_.tile([128, NT, D], f32)

            # uT+Wp gate mm1 of iteration 0 -> loaded first, fine-grained,
            # on the SP and Act queues so mm1 starts on the first chunks.
            last_ld = None
            for t0 in range(0, NT, 3):
                nc.sync.dma_start(
                    uT[:, t0 : t0 + 3, 0:128], uT_d[:, t0 : t0 + 3, 0:128]
                )
                nc.scalar.dma_start(Wp[:, t0 : t0 + 3, :], Wp_d[:, t0 : t0 + 3, :])
            for t0 in range(0, NT, 3):
                last_ld = nc.sync.dma_start(
                    uT[:, t0 : t0 + 3, 128:256], uT_d[:, t0 : t0 + 3, 128:256]
                )
            nc.gpsimd.memset(ones[:], 1.0)
            nc.gpsimd.memset(ones16[:], 1.0)
            # Warm the PE clock while the uT/Wp DMAs are in flight.
            if prewarm:
                pw_ps = ps_t.tile([128, 128], f32, name="pw", tag="wm")
                for k in range(prewarm):
                    nc.tensor.matmul(
                        pw_ps[:], ones16[:], ones16[:], start=True, stop=True
                    )
            # u_nat / J are not needed until mm2 (~15us in); gate them past
            # the last uT chunk so their DMA transfers stay off the
            # startup-critical DMA engine window.  They live on the Act
            # queue (idle until the first Square), keeping the Pool queue
            # free for the collectives.
            for h in range(NB):
                d = nc.scalar.dma_start(un[:, h, :], un_d[h])
                add_dep_helper(d.ins, last_ld.ins, sync=True,
                               reason="defer u_nat load past uT/Wp")
            dj = nc.scalar.dma_start(J[:], Jm_d[:])
            add_dep_helper(dj.ins, last_ld.ins, sync=True,
                           reason="defer J load past uT/Wp")

            for it in range(N_ITER * reps):
                rep, it = divmod(it, N_ITER)
                last = it == N_ITER - 1
                if it == 0:
                    # b==0 => c uniform: feed W directly, fold 1/(10*16)
                    # into the squash constants (s_dev = 10 * s_true).
                    CW = Wp
                    A2 = 100.0
                else:
                    # ---- batched softmax over d on COMPACT logits ----
                    # exp / den / recip / c are each ONE instruction over
                    # all 9 t-tiles; the o-broadcast happens inside the
                    # CW multiply via a 0-stride access pattern.
                    e = ip_.tile([128, NT, D], f32, name=f"e{rep}_{it}", tag="e")
                    den = ip_.tile([128, NT], f32, name=f"den{rep}_{it}", tag="den")
                    rec = ip_.tile([128, NT], f32, name=f"rc{rep}_{it}", tag="rc")
                    cc = ip_.tile([128, NT, D], f32, name=f"c{rep}_{it}", tag="c")
                    CW = ip_.tile([128, NT, DO], bf16, name=f"cw{rep}_{it}", tag="cw")
                    A2 = 1.0
                    nc.scalar.activation(e[:], blog[:], AF.Exp)
                    nc.vector.reduce_sum(
                        den[:].unsqueeze(2), e[:], axis=mybir.AxisListType.X
                    )
                    nc.vector.reciprocal(rec[:].unsqueeze(2), den[:].unsqueeze(2))
                    nc.vector.tensor_tensor(
                        cc[:], e[:],
                        rec[:].unsqueeze(2).broadcast_to([128, NT, D]),
                        op=ALU.mult,
                    )
                    for lo in range(0, NT, 3):
                        hi = lo + 3
                        nc.gpsimd.tensor_tensor(
                            CW[:, lo:hi, :].rearrange(
                                "p t (d o) -> p t d o", d=D, o=O
                            ),
                            Wp[:, lo:hi, :].rearrange(
                                "p t (d o) -> p t d o", d=D, o=O
                            ),
                            cc[:, lo:hi, :].unsqueeze(3).broadcast_to(
                                [128, hi - lo, D, O]
                            ),
                            op=ALU.mult,
                        )
                # ---- mm1: s_dev[b,(d,o)] = sum_(r,i) uT.T @ CW ----
                s_ps = ps_s.tile([128, NB, DO], f32, name=f"sps{rep}_{it}", tag="sps")
                s_sb = ip_.tile([128, NB, DO], f32, name=f"s{rep}_{it}", tag="s")
                for h in range(NB):
                    for t in range(NT):
                        nc.tensor.matmul(
                            s_ps[:, h, :],
                            uT[:, t, h * 128 : (h + 1) * 128],
                            CW[:, t, :],
                            start=(t == 0),
                            stop=(t == NT - 1),
                        )
                    nc.vector.tensor_copy(s_sb[:, h, :], s_ps[:, h, :])
                inb = dp_.tile([128, NB * DO], f32, name=f"inb{rep}_{it}", tag="inb")
                outb = dp_.tile(
                    [128, NB * DO], f32, name=f"outb{rep}_{it}", tag="outb",
                    addr_space="Shared",
                )
                nc.sync.dma_start(inb[:], s_sb[:].rearrange("p h f -> p (h f)"))
                # ---- AllReduce partial s over the 8 cores ----
                nc.gpsimd.collective_compute(
                    "AllReduce", ALU.add, replica_groups=rg,
                    ins=[inb.opt()], outs=[outb.opt()],
                )
                sf = ip_.tile([128, NB, DO], f32, name=f"sf{rep}_{it}", tag="sf")
                nc.sync.dma_start(sf[:].rearrange("p h f -> p (h f)"), outb[:])

                # ---- squash with global norm over the full batch ----
                # s_dev = A*s_true  =>  v = s_dev * sqrt(T)/(A^2 + T),
                # T = sum(s_dev^2).
                def emit_squash(rep=rep, it=it, sf=sf, A2=A2):
                    sqscr = sp_.tile(
                        [128, NB * DO], f32, name=f"sq{rep}_{it}", tag="sq"
                    )
                    ppsum = sp_.tile([128, 1], f32, name=f"pps{rep}_{it}", tag="pps")
                    nc.scalar.activation(
                        sqscr[:], sf[:].rearrange("p h f -> p (h f)"), AF.Square,
                        accum_out=ppsum[:],
                    )
                    # T broadcast to every partition via ones-matmul
                    T_ps = ps_t.tile([128, 1], f32, name=f"T{rep}_{it}", tag="wm")
                    nc.tensor.matmul(
                        T_ps[:], ones[:], ppsum[:], start=True, stop=True
                    )
                    q = sp_.tile([128, 1], f32, name=f"q{rep}_{it}", tag="q")
                    nc.vector.tensor_scalar_add(q[:], T_ps[:], A2)
                    qinv = sp_.tile([128, 1], f32, name=f"qi{rep}_{it}", tag="qi")
                    nc.vector.reciprocal(qinv[:], q[:])
                    rt = sp_.tile([128, 1], f32, name=f"rt{rep}_{it}", tag="rt")
                    nc.scalar.activation(rt[:], T_ps[:], AF.Sqrt)
                    g = sp_.tile([128, 1], f32, name=f"g{rep}_{it}", tag="g")
                    nc.vector.tensor_tensor(g[:], rt[:], qinv[:], op=ALU.mult)
                    return g

                if last:
                    g = emit_squash()
                    v_sb = ip_.tile([128, NB, DO], f32, name=f"v{rep}_{it}", tag="v")
                    nc.vector.tensor_scalar_mul(
                        v_sb[:].rearrange("p h f -> p (h f)"),
                        sf[:].rearrange("p h f -> p (h f)"),
                        g[:, 0:1],
                    )
                    nc.sync.dma_start(
                        v_out_d[:].rearrange("h p f -> p h f"), v_sb[:]
                    )
                else:
                    # ---- mm2 on sf directly (G = g*(u.T@sf)); the squash
                    # scalar g folds into the H multiply, so mm2 starts
                    # right at the AR return. ----
                    sf16 = ip_.tile(
                        [128, NB, DO], bf16, name=f"sf16{rep}_{it}", tag="sf16"
                    )
                    nc.vector.tensor_copy(
                        sf16[:].rearrange("p h f -> p (h f)"),
                        sf[:].rearrange("p h f -> p (h f)"),
                    )
                    Hred = ip_.tile([128, NT, D], f32, name=f"hr{rep}_{it}", tag="hr")
                    groups = [(0, 1), (1, 3), (3, 5), (5, 7), (7, 9)]

                    def emit_G(lo, hi, rep=rep, it=it, sf16=sf16):
                        G_ps = ps_g.tile(
                            [128, hi - lo, DO], f32,
                            name=f"G{rep}_{it}_{lo}", tag="G",
                        )
                        for k, t in enumerate(range(lo, hi)):
                            for h in range(NB):
                                nc.tensor.matmul(
                                    G_ps[:, k, :],
                                    un[:, h, t * 128 : (t + 1) * 128],
                                    sf16[:, h, :],
                                    start=(h == 0),
                                    stop=(h == NB - 1),
                                )
                        return G_ps

                    G_pre = emit_G(*groups[0])
                    g = emit_squash()
                    for gi, (lo, hi) in enumerate(groups):
                        n = hi - lo
                        G_ps = G_pre if gi == 0 else emit_G(lo, hi)
                        # Ht = (G * g) . Wp  (g: per-partition scalar)
                        Ht = sp_.tile(
                            [128, n, DO], f32, name=f"ht{rep}_{it}_{lo}", tag="ht"
                        )
                        nc.vector.scalar_tensor_tensor(
                            Ht[:], G_ps[:], g[:, 0:1], Wp[:, lo:hi, :],
                            op0=ALU.mult, op1=ALU.mult,
                        )
                        nc.vector.reduce_sum(
                            Hred[:, lo:hi, :],
                            Ht[:].rearrange("p t (d o) -> p t d o", d=D, o=O),
                            axis=mybir.AxisListType.X,
                        )
                        # i-sum + broadcast via the block-diag ones matmul
                        bd_ps = ps_bd.tile(
                            [128, n * D], f32, name=f"bd{rep}_{it}_{lo}", tag="bd"
                        )
                        nc.tensor.matmul(
                            bd_ps[:], J[:], Hred[:, lo:hi, :], start=True, stop=True
                        )
                        bd_v = bd_ps[:].rearrange("p (t d) -> p t d", t=n, d=D)
                        if it == 0:
                            # blog starts at 0: first update is a plain copy
                            nc.vector.tensor_copy(blog[:, lo:hi, :], bd_v)
                        else:
                            nc.vector.tensor_tensor(
                                blog[:, lo:hi, :], blog[:, lo:hi, :], bd_v,
                                op=ALU.add,
                            )

    _split_excess_waits(nc, 1)
    return nc


_NC_CACHE = {}


def _get_nc(reps=1):
    key = (reps,)
    if key not in _NC_CACHE:
        _NC_CACHE[key] = _build_nc(reps=reps)
    return _NC_CACHE[key]


def _prep_core_inputs(u, W, c):
    r0, r1 = c * RL, (c + 1) * RL
    u2 = np.ascontiguousarray(u[:, r0:r1, :]).reshape(B, KRI)
    u_nat = np.ascontiguousarray(u2.reshape(NB, 128, KRI)).astype(ml_dtypes.bfloat16)
    uT = np.ascontiguousarray(
        np.ascontiguousarray(u2.T).reshape(NT, 128, B).transpose(1, 0, 2)
    ).astype(ml_dtypes.bfloat16)
    Wp2 = np.ascontiguousarray(W[0, r0:r1].transpose(0, 3, 1, 2)).reshape(KRI, DO)
    Wp = np.ascontiguousarray(
        Wp2.reshape(NT, 128, DO).transpose(1, 0, 2)
    ).astype(ml_dtypes.bfloat16)
    return {"u_nat": u_nat, "uT": uT, "Wp": Wp}


def kernel(u, W, _trace=False, _reps=1):
    u = np.asarray(u, dtype=np.float32)
    W = np.asarray(W, dtype=np.float32)
    assert u.shape == (B, R, I_CH) and W.shape == (1, R, D, O, I_CH)
    Jm = np.kron(np.eye(16, dtype=np.float32), np.ones((8, 8), np.float32))
    in_maps = []
    for c in range(N_CORES):
        m = _prep_core_inputs(u, W, c)
        m["Jm"] = Jm
        in_maps.append(m)
    nc = _get_nc(_reps)
    res = run_bass_kernel_spmd(
        nc, in_maps, core_ids=list(range(N_CORES)), trace=_trace
    )
    v = res.results[0]["v_out"].reshape(B, D, O).astype(np.float32)
    if _trace:
        return v, res
    return v


# revision 10
# speedup vs baseline: 1.0599x; 1.0599x over previous
"""DigitCapsule dynamic-routing kernel for 8 Trainium2 NeuronCores.

Key restructuring: u_hat (B,R,D,O) = 188 MB is NEVER materialized.
  s[b,(d,o)]  = sum_{(r,i)} (c[r,d]*W[r,d,o,i]) * u[b,r,i]      (matmul over (r,i))
  b_upd[r,d]  = sum_{i,o} W[r,d,o,i] * G[(r,i),(d,o)],
  G[(r,i),(d,o)] = sum_b u[b,(r,i)] * v[b,(d,o)]                 (matmul over b)

Sharding: route nodes R=1152 are split 144/core across 8 cores.  Softmax
(over d) and the b-logit update are then fully local; the only collective
is one AllReduce of the partial s per routing iteration (3 total).

Perf notes (v2):
  * All PE operands are bf16 (fp32 matmuls cost 4 cycles/row vs bf16's 1);
    PSUM accumulation and the AllReduce payload stay fp32, as does all
    squash / logit math, so the only precision loss is the bf16 input
    rounding (~0.3% relative, vs a 2e-2 gate).
  * The gpsimd (Pool) queue carries ONLY the collectives plus SBUF-only
    elementwise offload (Hred / CW); u_nat+J loads moved to the DVE queue
    and gated past the uT/Wp loads so neither the Pool sequencer nor the
    serialized DMA transfer engine delays the iteration-0 AllReduce.
  * Softmax is batched: one Exp over all 9 t-tiles + one reduce + one
    reciprocal + one broadcast multiply, instead of 9 per-t Act chains.
  * s goes PSUM -> (per-half DVE drain) -> one [128,320] DMA to the
    collective input; sf comes back in the same layout (no rearrange).
The device tracks s_dev = A*s_true (A=1 normally; iteration 0 skips the
softmax entirely, feeding W straight to mm1, so A = 10 there) and corrects
inside squash: v = s_dev * sqrt(T)/(A^2 + T) with T = sum(s_dev^2).
Iterations 0-1 never materialize v at all: mm2 consumes the AllReduce
output directly and the squash scalar g = sqrt(T)/(A^2+T) folds into the
W*G multiply.
"""

import ml_dtypes
import numpy as np

import concourse.bass as bass
import concourse.mybir as mybir
import concourse.tile as tile
from concourse.bass_utils import run_bass_kernel_spmd
from concourse.tile import add_dep_helper

N_CORES = 8
B, R, D, O, I_CH = 256, 1152, 10, 16, 8
RL = R // N_CORES           # 144 route nodes per core
KRI = RL * I_CH             # 1152 = (r,i) contraction length per core
NT = KRI // 128             # 9 partition tiles of (r,i)
DO = D * O                  # 160
NB = B // 128               # 2 batch halves
N_ITER = 3

f32 = mybir.dt.float32
bf16 = mybir.dt.bfloat16
ALU = mybir.AluOpType
AF = mybir.ActivationFunctionType

_ws_ctr = [0]


def _split_excess_waits(nc, max_waits=1):
    """Walrus in this container only lowers one sync-wait per instruction.
    Hoist excess waits onto NOPs inserted before the instruction on the
    same engine (same-order execution => identical semantics)."""
    n_split = 0
    for f in nc.m.functions:
        for bb in f.blocks:
            out = []
            changed = False
            for ins in bb.instructions:
                si = ins.sync_info
                waits = list(si.on_wait) if (si is not None and si.on_wait) else []
                if len(waits) > max_waits:
                    changed = True
                    n_split += 1
                    head, rest = waits[:-max_waits], waits[-max_waits:]
                    while head:
                        chunk, head = head[:max_waits], head[max_waits:]
                        _ws_ctr[0] += 1
                        nop = mybir.InstNoOp(name=f"I-ws{_ws_ctr[0]}")
                        nop.engine = ins.engine
                        nop.sync_info = mybir.SyncInfo(on_wait=chunk, on_update=[])
                        out.append(nop)
                    ins.sync_info = mybir.SyncInfo(
                        on_wait=rest,
                        on_update=list(si.on_update) if si.on_update else [],
                    )
                out.append(ins)
            if changed:
                bb.instructions = out
    return n_split


def _build_nc(reps=1, prewarm=10):
    nc = bass.Bass(
        "TRN2", target_bir_lowering=False, debug=False, num_devices=N_CORES
    )
    un_d = nc.dram_tensor("u_nat", [NB, 128, KRI], bf16, kind="ExternalInput")
    uT_d = nc.dram_tensor("uT", [128, NT, B], bf16, kind="ExternalInput")
    Wp_d = nc.dram_tensor("Wp", [128, NT, DO], bf16, kind="ExternalInput")
    Jm_d = nc.dram_tensor("Jm", [128, 128], f32, kind="ExternalInput")
    v_out_d = nc.dram_tensor("v_out", [NB, 128, DO], f32, kind="ExternalOutput")

    rg = [list(range(N_CORES))]

    with tile.TileContext(nc) as tc:
        with (
            tc.tile_pool(name="persist", bufs=1) as pp_,
            tc.tile_pool(name="iter", bufs=2) as ip_,
            tc.tile_pool(name="small", bufs=2) as sp_,
            tc.tile_pool(name="dram", bufs=2, space="DRAM") as dp_,
            tc.tile_pool(name="ps_s", bufs=2, space="PSUM") as ps_s,
            tc.tile_pool(name="ps_g", bufs=2, space="PSUM") as ps_g,
            tc.tile_pool(name="ps_bd", bufs=2, space="PSUM") as ps_bd,
            tc.tile_pool(name="ps_t", bufs=1, space="PSUM") as ps_t,
        ):
            # ---- persistent tensors ----
            un = pp_.tile([128, NB, KRI], bf16)
            uT = pp_.tile([128, NT, B], bf16)
            Wp = pp_.tile([128, NT, DO], bf16)
            J = pp_.tile([128, 128], f32)
            ones = pp_.tile([128, 128], f32)
            ones16 = pp_.tile([128, 128], bf16)
            blog = pp_.tile([128, NT, D], f32)

            # uT+Wp gate mm1 of iteration 0 -> loaded first, fine-grained,
            # on the SP and Act queues so mm1 starts on the first chunks.
            # Chunks keep the full 256-wide b axis so the innermost
            # contiguous run is >= 512B (avoids the 2x DMA penalty).
            last_ld = None
            for t0 in range(0, NT, 3):
                last_ld = nc.sync.dma_start(
                    uT[:, t0 : t0 + 3, :], uT_d[:, t0 : t0 + 3, :]
                )
                nc.scalar.dma_start(Wp[:, t0 : t0 + 3, :], Wp_d[:, t0 : t0 + 3, :])
            nc.gpsimd.memset(ones[:], 1.0)
            nc.gpsimd.memset(ones16[:], 1.0)
            # Warm the PE clock while the uT/Wp DMAs are in flight.
            if prewarm:
                pw_ps = ps_t.tile([128, 128], f32, name="pw", tag="wm")
                for k in range(prewarm):
                    nc.tensor.matmul(
                        pw_ps[:], ones16[:], ones16[:], start=True, stop=True
                    )
            # u_nat / J are not needed until mm2 (~15us in).  They are
            # emitted on the Act queue (idle until the first Square) and
            # gated on iteration 0's collective-input DMA so their
            # transfers slide into the AllReduce window instead of
            # delaying it on the serialized DMA transfer engine.
            deferred = [False]

            def _emit_deferred_loads(anchor):
                if deferred[0]:
                    return
                deferred[0] = True
                for h in range(NB):
                    d = nc.scalar.dma_start(un[:, h, :], un_d[h])
                    add_dep_helper(d.ins, anchor.ins, sync=True,
                                   reason="defer u_nat past AR0 input")
                dj = nc.scalar.dma_start(J[:], Jm_d[:])
                add_dep_helper(dj.ins, anchor.ins, sync=True,
                               reason="defer J past AR0 input")

            for it in range(N_ITER * reps):
                rep, it = divmod(it, N_ITER)
                last = it == N_ITER - 1
                if it == 0:
                    # b==0 => c uniform: feed W directly, fold 1/(10*16)
                    # into the squash constants (s_dev = 10 * s_true).
                    CW = Wp
                    A2 = 100.0
                else:
                    # ---- batched softmax over d on COMPACT logits ----
                    # exp / den / recip / c are each ONE instruction over
                    # all 9 t-tiles; the o-broadcast happens inside the
                    # CW multiply via a 0-stride access pattern.
                    e = ip_.tile([128, NT, D], f32, name=f"e{rep}_{it}", tag="e")
                    den = ip_.tile([128, NT], f32, name=f"den{rep}_{it}", tag="den")
                    rec = ip_.tile([128, NT], f32, name=f"rc{rep}_{it}", tag="rc")
                    cc = ip_.tile([128, NT, D], f32, name=f"c{rep}_{it}", tag="c")
                    CW = ip_.tile([128, NT, DO], bf16, name=f"cw{rep}_{it}", tag="cw")
                    A2 = 1.0
                    nc.scalar.activation(e[:], blog[:], AF.Exp)
                    nc.vector.reduce_sum(
                        den[:].unsqueeze(2), e[:], axis=mybir.AxisListType.X
                    )
                    nc.vector.reciprocal(rec[:].unsqueeze(2), den[:].unsqueeze(2))
                    nc.vector.tensor_tensor(
                        cc[:], e[:],
                        rec[:].unsqueeze(2).broadcast_to([128, NT, D]),
                        op=ALU.mult,
                    )
                    for lo in range(0, NT, 3):
                        hi = lo + 3
                        nc.vector.tensor_tensor(
                            CW[:, lo:hi, :].rearrange(
                                "p t (d o) -> p t d o", d=D, o=O
                            ),
                            Wp[:, lo:hi, :].rearrange(
                                "p t (d o) -> p t d o", d=D, o=O
                            ),
                            cc[:, lo:hi, :].unsqueeze(3).broadcast_to(
                                [128, hi - lo, D, O]
                            ),
                            op=ALU.mult,
                        )
                # ---- mm1: s_dev[b,(d,o)] = sum_(r,i) uT.T @ CW ----
                s_ps = ps_s.tile([128, NB, DO], f32, name=f"sps{rep}_{it}", tag="sps")
                s_sb = ip_.tile([128, NB, DO], f32, name=f"s{rep}_{it}", tag="s")
                for h in range(NB):
                    for t in range(NT):
                        nc.tensor.matmul(
                            s_ps[:, h, :],
                            uT[:, t, h * 128 : (h + 1) * 128],
                            CW[:, t, :],
                            start=(t == 0),
                            stop=(t == NT - 1),
                        )
                    nc.vector.tensor_copy(s_sb[:, h, :], s_ps[:, h, :])
                inb = dp_.tile([128, NB * DO], f32, name=f"inb{rep}_{it}", tag="inb")
                outb = dp_.tile(
                    [128, NB * DO], f32, name=f"outb{rep}_{it}", tag="outb",
                    addr_space="Shared",
                )
                inb_dma = nc.sync.dma_start(
                    inb[:], s_sb[:].rearrange("p h f -> p (h f)")
                )
                if it == 0 and rep == 0:
                    _emit_deferred_loads(inb_dma)
                # ---- AllReduce partial s over the 8 cores ----
                nc.gpsimd.collective_compute(
                    "AllReduce", ALU.add, replica_groups=rg,
                    ins=[inb.opt()], outs=[outb.opt()],
                )
                sf = ip_.tile([128, NB, DO], f32, name=f"sf{rep}_{it}", tag="sf")
                nc.sync.dma_start(sf[:].rearrange("p h f -> p (h f)"), outb[:])

                # ---- squash with global norm over the full batch ----
                # s_dev = A*s_true  =>  v = s_dev * sqrt(T)/(A^2 + T),
                # T = sum(s_dev^2).
                def emit_squash(rep=rep, it=it, sf=sf, A2=A2):
                    sqscr = sp_.tile(
                        [128, NB * DO], f32, name=f"sq{rep}_{it}", tag="sq"
                    )
                    ppsum = sp_.tile([128, 1], f32, name=f"pps{rep}_{it}", tag="pps")
                    nc.scalar.activation(
                        sqscr[:], sf[:].rearrange("p h f -> p (h f)"), AF.Square,
                        accum_out=ppsum[:],
                    )
                    # T broadcast to every partition via ones-matmul
                    T_ps = ps_t.tile([128, 1], f32, name=f"T{rep}_{it}", tag="wm")
                    nc.tensor.matmul(
                        T_ps[:], ones[:], ppsum[:], start=True, stop=True
                    )
                    q = sp_.tile([128, 1], f32, name=f"q{rep}_{it}", tag="q")
                    nc.vector.tensor_scalar_add(q[:], T_ps[:], A2)
                    qinv = sp_.tile([128, 1], f32, name=f"qi{rep}_{it}", tag="qi")
                    nc.vector.reciprocal(qinv[:], q[:])
                    rt = sp_.tile([128, 1], f32, name=f"rt{rep}_{it}", tag="rt")
                    nc.scalar.activation(rt[:], T_ps[:], AF.Sqrt)
                    g = sp_.tile([128, 1], f32, name=f"g{rep}_{it}", tag="g")
                    nc.vector.tensor_tensor(g[:], rt[:], qinv[:], op=ALU.mult)
                    return g

                if last:
                    g = emit_squash()
                    v_sb = ip_.tile([128, NB, DO], f32, name=f"v{rep}_{it}", tag="v")
                    nc.vector.tensor_scalar_mul(
                        v_sb[:].rearrange("p h f -> p (h f)"),
                        sf[:].rearrange("p h f -> p (h f)"),
                        g[:, 0:1],
                    )
                    nc.sync.dma_start(
                        v_out_d[:].rearrange("h p f -> p h f"), v_sb[:]
                    )
                else:
                    # ---- mm2 on sf directly (G = g*(u.T@sf)); the squash
                    # scalar g folds into the H multiply, so mm2 starts
                    # right at the AR return. ----
                    sf16 = ip_.tile(
                        [128, NB, DO], bf16, name=f"sf16{rep}_{it}", tag="sf16"
                    )
                    nc.vector.tensor_copy(
                        sf16[:].rearrange("p h f -> p (h f)"),
                        sf[:].rearrange("p h f -> p (h f)"),
                    )
                    Hred = ip_.tile([128, NT, D], f32, name=f"hr{rep}_{it}", tag="hr")
                    groups = [(0, 2), (2, 4), (4, 6), (6, 8), (8, 9)]

                    def emit_G(lo, hi, rep=rep, it=it, sf16=sf16):
                        G_ps = ps_g.tile(
                            [128, hi - lo, DO], f32,
                            name=f"G{rep}_{it}_{lo}", tag="G",
                        )
                        for k, t in enumerate(range(lo, hi)):
                            for h in range(NB):
                                nc.tensor.matmul(
                                    G_ps[:, k, :],
                                    un[:, h, t * 128 : (t + 1) * 128],
                                    sf16[:, h, :],
                                    start=(h == 0),
                                    stop=(h == NB - 1),
                                )
                        return G_ps

                    G_pre = emit_G(*groups[0])
                    g = emit_squash()
                    for gi, (lo, hi) in enumerate(groups):
                        n = hi - lo
                        G_ps = G_pre if gi == 0 else emit_G(lo, hi)
                        # Ht = G . Wp; the squash scalar g folds into the
                        # blog update below so the H chain never waits on
                        # the squash-scalar latency.
                        Ht = sp_.tile(
                            [128, n, DO], f32, name=f"ht{rep}_{it}_{lo}", tag="ht"
                        )
                        nc.vector.tensor_tensor(
                            Ht[:], G_ps[:], Wp[:, lo:hi, :], op=ALU.mult
                        )
                        nc.vector.reduce_sum(
                            Hred[:, lo:hi, :],
                            Ht[:].rearrange("p t (d o) -> p t d o", d=D, o=O),
                            axis=mybir.AxisListType.X,
                        )
                        # i-sum + broadcast via the block-diag ones matmul
                        bd_ps = ps_bd.tile(
                            [128, n * D], f32, name=f"bd{rep}_{it}_{lo}", tag="bd"
                        )
                        nc.tensor.matmul(
                            bd_ps[:], J[:], Hred[:, lo:hi, :], start=True, stop=True
                        )
                        bd_v = bd_ps[:].rearrange("p (t d) -> p t d", t=n, d=D)
                        if it == 0:
                            # blog starts at 0: first update is g*bd
                            nc.vector.tensor_scalar_mul(
                                blog[:, lo:hi, :], bd_v, g[:, 0:1]
                            )
                        else:
                            nc.vector.scalar_tensor_tensor(
                                blog[:, lo:hi, :], bd_v, g[:, 0:1],
                                blog[:, lo:hi, :], op0=ALU.mult, op1=ALU.add,
                            )

    _split_excess_waits(nc, 1)
    return nc


_NC_CACHE = {}


def _get_nc(reps=1):
    key = (reps,)
    if key not in _NC_CACHE:
        _NC_CACHE[key] = _build_nc(reps=reps)
    return _NC_CACHE[key]


def _prep_core_inputs(u, W, c):
    r0, r1 = c * RL, (c + 1) * RL
    u2 = np.ascontiguousarray(u[:, r0:r1, :]).reshape(B, KRI)
    u_nat = np.ascontiguousarray(u2.reshape(NB, 128, KRI)).astype(ml_dtypes.bfloat16)
    uT = np.ascontiguousarray(
        np.ascontiguousarray(u2.T).reshape(NT, 128, B).transpose(1, 0, 2)
    ).astype(ml_dtypes.bfloat16)
    Wp2 = np.ascontiguousarray(W[0, r0:r1].transpose(0, 3, 1, 2)).reshape(KRI, DO)
    Wp = np.ascontiguousarray(
        Wp2.reshape(NT, 128, DO).transpose(1, 0, 2)
    ).astype(ml_dtypes.bfloat16)
    return {"u_nat": u_nat, "uT": uT, "Wp": Wp}


def kernel(u, W, _trace=False, _reps=1):
    u = np.asarray(u, dtype=np.float32)
    W = np.asarray(W, dtype=np.float32)
    assert u.shape == (B, R, I_CH) and W.shape == (1, R, D, O, I_CH)
    Jm = np.kron(np.eye(16, dtype=np.float32), np.ones((8, 8), np.float32))
    in_maps = []
    for c in range(N_CORES):
        m = _prep_core_inputs(u, W, c)
        m["Jm"] = Jm
        in_maps.append(m)
    nc = _get_nc(_reps)
    res = run_bass_kernel_spmd(
        nc, in_maps, core_ids=list(range(N_CORES)), trace=_trace
    )
    v = res.results[0]["v_out"].reshape(B, D, O).astype(np.float32)
    if _trace:
        return v, res
    return v


# revision 12
# speedup vs baseline: 1.1426x; 1.0780x over previous
"""DigitCapsule dynamic-routing kernel for 8 Trainium2 NeuronCores.

Key restructuring: u_hat (B,R,D,O) = 188 MB is NEVER materialized.
  s[b,(d,o)]  = sum_{(r,i)} (c[r,d]*W[r,d,o,i]) * u[b,r,i]      (matmul over (r,i))
  b_upd[r,d]  = sum_{i,o} W[r,d,o,i] * G[(r,i),(d,o)],
  G[(r,i),(d,o)] = sum_b u[b,(r,i)] * v[b,(d,o)]                 (matmul over b)

Sharding: route nodes R=1152 are split 144/core across 8 cores.  Softmax
(over d) and the b-logit update are then fully local; the only collective
is one AllReduce of the partial s per routing iteration (3 total).

Perf notes (v4):
  * All PE operands are bf16 (fp32 matmuls cost 4 cycles/row vs bf16's 1).
    The AllReduce payload is bf16 too, so its output feeds mm2 directly.
    PSUM accumulation and all squash / logit math stay fp32.
  * The squash scalar g folds into the blog update (not the W*G multiply),
    so the mm2 chain never waits on the squash-scalar latency.
  * gpsimd (Pool) queue: collectives + the collective-input DMA (no HWDGE
    round-trip) + one CW chunk; u_nat+J ride the Act queue gated past the
    uT/Wp loads so nothing contends with the AR window on the serialized
    DMA transfer engine.
  * Softmax is batched (one Exp / reduce / reciprocal / multiply), CW is
    produced in 3 chunks split DVE/Pool/DVE, and mm1 consumes them as
    they land.
  * mm2's G drains PSUM->bf16 on the idle Act engine so the W*G multiply
    runs in the DVE's 2x 16-bit mode; the o-reduce accumulates in fp32.
The device tracks s_dev = A*s_true (A=1 normally; iteration 0 skips the
softmax entirely, feeding W straight to mm1, so A = 10 there) and corrects
inside squash: v = s_dev * sqrt(T)/(A^2 + T) with T = sum(s_dev^2).
"""

import ml_dtypes
import numpy as np

import concourse.bass as bass
import concourse.mybir as mybir
import concourse.tile as tile
from concourse.bass_utils import run_bass_kernel_spmd
from concourse.tile import add_dep_helper

N_CORES = 8
B, R, D, O, I_CH = 256, 1152, 10, 16, 8
RL = R // N_CORES           # 144 route nodes per core
KRI = RL * I_CH             # 1152 = (r,i) contraction length per core
NT = KRI // 128             # 9 partition tiles of (r,i)
DO = D * O                  # 160
NB = B // 128               # 2 batch halves
N_ITER = 3

f32 = mybir.dt.float32
bf16 = mybir.dt.bfloat16
ALU = mybir.AluOpType
AF = mybir.ActivationFunctionType

_ws_ctr = [0]


def _split_excess_waits(nc, max_waits=1):
    """Walrus in this container only lowers one sync-wait per instruction.
    Hoist excess waits onto NOPs inserted before the instruction on the
    same engine (same-order execution => identical semantics)."""
    n_split = 0
    for f in nc.m.functions:
        for bb in f.blocks:
            out = []
            changed = False
            for ins in bb.instructions:
                si = ins.sync_info
                waits = list(si.on_wait) if (si is not None and si.on_wait) else []
                if len(waits) > max_waits:
                    changed = True
                    n_split += 1
                    head, rest = waits[:-max_waits], waits[-max_waits:]
                    while head:
                        chunk, head = head[:max_waits], head[max_waits:]
                        _ws_ctr[0] += 1
                        nop = mybir.InstNoOp(name=f"I-ws{_ws_ctr[0]}")
                        nop.engine = ins.engine
                        nop.sync_info = mybir.SyncInfo(on_wait=chunk, on_update=[])
                        out.append(nop)
                    ins.sync_info = mybir.SyncInfo(
                        on_wait=rest,
                        on_update=list(si.on_update) if si.on_update else [],
                    )
                out.append(ins)
            if changed:
                bb.instructions = out
    return n_split


def _build_nc(reps=1, prewarm=10):
    nc = bass.Bass(
        "TRN2", target_bir_lowering=False, debug=False, num_devices=N_CORES
    )
    un_d = nc.dram_tensor("u_nat", [NB, 128, KRI], bf16, kind="ExternalInput")
    uT_d = nc.dram_tensor("uT", [128, NT, B], bf16, kind="ExternalInput")
    Wp_d = nc.dram_tensor("Wp", [128, NT, DO], bf16, kind="ExternalInput")
    Jm_d = nc.dram_tensor("Jm", [128, 128], f32, kind="ExternalInput")
    v_out_d = nc.dram_tensor("v_out", [NB, 128, DO], f32, kind="ExternalOutput")

    rg = [list(range(N_CORES))]

    with tile.TileContext(nc) as tc:
        with (
            tc.tile_pool(name="persist", bufs=1) as pp_,
            tc.tile_pool(name="iter", bufs=2) as ip_,
            tc.tile_pool(name="small", bufs=2) as sp_,
            tc.tile_pool(name="dram", bufs=2, space="DRAM") as dp_,
            tc.tile_pool(name="ps_s", bufs=1, space="PSUM") as ps_s,
            tc.tile_pool(name="ps_g", bufs=2, space="PSUM") as ps_g,
            tc.tile_pool(name="ps_bd", bufs=2, space="PSUM") as ps_bd,
            tc.tile_pool(name="ps_t", bufs=1, space="PSUM") as ps_t,
        ):
            # ---- persistent tensors ----
            un = pp_.tile([128, NB, KRI], bf16)
            uT = pp_.tile([128, NT, B], bf16)
            Wp = pp_.tile([128, NT, DO], bf16)
            J = pp_.tile([128, 128], f32)
            ones = pp_.tile([128, 128], f32)
            ones16 = pp_.tile([128, 128], bf16)
            blog = pp_.tile([128, NT, D], f32)

            # uT+Wp gate mm1 of iteration 0 -> loaded first, fine-grained,
            # on the SP and Act queues so mm1 starts on the first chunks.
            # Chunks keep the full 256-wide b axis so the innermost
            # contiguous run is >= 512B (avoids the 2x DMA penalty).
            last_ld = None
            for t0 in range(0, NT, 3):
                last_ld = nc.sync.dma_start(
                    uT[:, t0 : t0 + 3, :], uT_d[:, t0 : t0 + 3, :]
                )
                nc.scalar.dma_start(Wp[:, t0 : t0 + 3, :], Wp_d[:, t0 : t0 + 3, :])
            nc.gpsimd.memset(ones[:], 1.0)
            nc.gpsimd.memset(ones16[:], 1.0)
            # Warm the PE clock while the uT/Wp DMAs are in flight.
            if prewarm:
                pw_ps = ps_t.tile([128, 128], f32, name="pw", tag="wm")
                for k in range(prewarm):
                    nc.tensor.matmul(
                        pw_ps[:], ones16[:], ones16[:], start=True, stop=True
                    )
            # u_nat / J are not needed until mm2 (~20us in); gate them on
            # the last uT chunk so their transfers run after the uT/Wp
            # window but before the AR0 input hits the serialized DMA
            # transfer engine.  They ride the Act queue (idle until the
            # first Square); the Pool queue stays clear for collectives.
            half = KRI // 2
            for h in range(NB):
                for q0 in range(0, KRI, half):
                    d = nc.scalar.dma_start(
                        un[:, h, q0 : q0 + half], un_d[h, :, q0 : q0 + half]
                    )
                    add_dep_helper(d.ins, last_ld.ins, sync=True,
                                   reason="defer u_nat past uT/Wp")
            dj = nc.scalar.dma_start(J[:], Jm_d[:])
            add_dep_helper(dj.ins, last_ld.ins, sync=True,
                           reason="defer J past uT/Wp")

            for it in range(N_ITER * reps):
                rep, it = divmod(it, N_ITER)
                last = it == N_ITER - 1
                if it == 0:
                    # b==0 => c uniform: feed W directly, fold 1/(10*16)
                    # into the squash constants (s_dev = 10 * s_true).
                    CW = Wp
                    A2 = 100.0
                else:
                    # ---- batched softmax over d on COMPACT logits ----
                    e = ip_.tile([128, NT, D], f32, name=f"e{rep}_{it}", tag="e")
                    den = ip_.tile([128, NT], f32, name=f"den{rep}_{it}", tag="den")
                    rec = ip_.tile([128, NT], f32, name=f"rc{rep}_{it}", tag="rc")
                    cc = ip_.tile([128, NT, D], f32, name=f"c{rep}_{it}", tag="c")
                    CW = ip_.tile([128, NT, DO], bf16, name=f"cw{rep}_{it}", tag="cw")
                    A2 = 1.0
                    nc.scalar.activation(e[:], blog[:], AF.Exp)
                    nc.vector.reduce_sum(
                        den[:].unsqueeze(2), e[:], axis=mybir.AxisListType.X
                    )
                    nc.vector.reciprocal(rec[:].unsqueeze(2), den[:].unsqueeze(2))
                    nc.vector.tensor_tensor(
                        cc[:], e[:],
                        rec[:].unsqueeze(2).broadcast_to([128, NT, D]),
                        op=ALU.mult,
                    )
                    # CW chunks: DVE / Pool / DVE run concurrently; mm1
                    # consumes them as they land.
                    for ci, lo in enumerate(range(0, NT, 3)):
                        hi = lo + 3
                        eng = nc.gpsimd if ci == 1 else nc.vector
                        eng.tensor_tensor(
                            CW[:, lo:hi, :].rearrange(
                                "p t (d o) -> p t d o", d=D, o=O
                            ),
                            Wp[:, lo:hi, :].rearrange(
                                "p t (d o) -> p t d o", d=D, o=O
                            ),
                            cc[:, lo:hi, :].unsqueeze(3).broadcast_to(
                                [128, hi - lo, D, O]
                            ),
                            op=ALU.mult,
                        )
                # ---- mm1: s_dev[b,(d,o)] = sum_(r,i) uT.T @ CW ----
                s_sb = ip_.tile([128, NB, DO], bf16, name=f"s{rep}_{it}", tag="s")
                for h in range(NB):
                    s_ps = ps_s.tile(
                        [128, DO], f32, name=f"sps{rep}_{it}_{h}", tag=f"sps{h}"
                    )
                    for t in range(NT):
                        nc.tensor.matmul(
                            s_ps[:],
                            uT[:, t, h * 128 : (h + 1) * 128],
                            CW[:, t, :],
                            start=(t == 0),
                            stop=(t == NT - 1),
                        )
                    nc.vector.tensor_copy(s_sb[:, h, :], s_ps[:])
                inb = dp_.tile([128, NB * DO], bf16, name=f"inb{rep}_{it}", tag="inb")
                outb = dp_.tile(
                    [128, NB * DO], bf16, name=f"outb{rep}_{it}", tag="outb",
                    addr_space="Shared",
                )
                # collective-input DMA on the Pool queue: no HWDGE hop and
                # the chain into the collective stays on one sequencer.
                nc.gpsimd.dma_start(inb[:], s_sb[:].rearrange("p h f -> p (h f)"))
                # ---- AllReduce partial s (bf16) over the 8 cores ----
                nc.gpsimd.collective_compute(
                    "AllReduce", ALU.add, replica_groups=rg,
                    ins=[inb.opt()], outs=[outb.opt()],
                )
                sf = ip_.tile([128, NB, DO], bf16, name=f"sf{rep}_{it}", tag="sf")
                nc.sync.dma_start(sf[:].rearrange("p h f -> p (h f)"), outb[:])

                # ---- squash with global norm over the full batch ----
                # s_dev = A*s_true  =>  v = s_dev * sqrt(T)/(A^2 + T),
                # T = sum(s_dev^2).
                def emit_squash(rep=rep, it=it, sf=sf, A2=A2):
                    sqscr = sp_.tile(
                        [128, NB * DO], f32, name=f"sq{rep}_{it}", tag="sq"
                    )
                    ppsum = sp_.tile([128, 1], f32, name=f"pps{rep}_{it}", tag="pps")
                    nc.scalar.activation(
                        sqscr[:], sf[:].rearrange("p h f -> p (h f)"), AF.Square,
                        accum_out=ppsum[:],
                    )
                    # T broadcast to every partition via ones-matmul
                    T_ps = ps_t.tile([128, 1], f32, name=f"T{rep}_{it}", tag="wm")
                    nc.tensor.matmul(
                        T_ps[:], ones[:], ppsum[:], start=True, stop=True
                    )
                    q = sp_.tile([128, 1], f32, name=f"q{rep}_{it}", tag="q")
                    nc.vector.tensor_scalar_add(q[:], T_ps[:], A2)
                    qinv = sp_.tile([128, 1], f32, name=f"qi{rep}_{it}", tag="qi")
                    nc.vector.reciprocal(qinv[:], q[:])
                    rt = sp_.tile([128, 1], f32, name=f"rt{rep}_{it}", tag="rt")
                    nc.scalar.activation(rt[:], T_ps[:], AF.Sqrt)
                    g = sp_.tile([128, 1], f32, name=f"g{rep}_{it}", tag="g")
                    nc.vector.tensor_tensor(g[:], rt[:], qinv[:], op=ALU.mult)
                    return g

                if last:
                    g = emit_squash()
                    v_sb = ip_.tile([128, NB, DO], f32, name=f"v{rep}_{it}", tag="v")
                    nc.vector.tensor_scalar_mul(
                        v_sb[:].rearrange("p h f -> p (h f)"),
                        sf[:].rearrange("p h f -> p (h f)"),
                        g[:, 0:1],
                    )
                    nc.sync.dma_start(
                        v_out_d[:].rearrange("h p f -> p h f"), v_sb[:]
                    )
                else:
                    # ---- mm2 on sf directly (G = u.T@sf; the squash
                    # scalar g folds into the blog update) ----
                    Hred = ip_.tile([128, NT, D], f32, name=f"hr{rep}_{it}", tag="hr")
                    groups = [(0, 2), (2, 4), (4, 6), (6, 8), (8, 9)]

                    def emit_G(lo, hi, rep=rep, it=it, sf=sf):
                        G_ps = ps_g.tile(
                            [128, hi - lo, DO], f32,
                            name=f"G{rep}_{it}_{lo}", tag="G",
                        )
                        for k, t in enumerate(range(lo, hi)):
                            for h in range(NB):
                                nc.tensor.matmul(
                                    G_ps[:, k, :],
                                    un[:, h, t * 128 : (t + 1) * 128],
                                    sf[:, h, :],
                                    start=(h == 0),
                                    stop=(h == NB - 1),
                                )
                        return G_ps

                    G_pre = emit_G(*groups[0])
                    g = emit_squash()
                    for gi, (lo, hi) in enumerate(groups):
                        n = hi - lo
                        G_ps = G_pre if gi == 0 else emit_G(lo, hi)
                        if gi < len(groups) - 1:
                            # drain G to bf16 on the (idle) Act engine so
                            # the W*G multiply runs in DVE 2x 16-bit mode
                            G16 = sp_.tile(
                                [128, n, DO], bf16,
                                name=f"g16{rep}_{it}_{lo}", tag="g16",
                            )
                            nc.scalar.activation(G16[:], G_ps[:], AF.Copy)
                            Ht = sp_.tile(
                                [128, n, DO], bf16,
                                name=f"ht{rep}_{it}_{lo}", tag="ht",
                            )
                            nc.vector.tensor_tensor(
                                Ht[:], G16[:], Wp[:, lo:hi, :], op=ALU.mult
                            )
                        else:
                            # last (small) group: skip the Act hop, lowest
                            # latency into the softmax tail
                            Ht = sp_.tile(
                                [128, n, DO], f32,
                                name=f"ht{rep}_{it}_{lo}", tag="ht",
                            )
                            nc.vector.tensor_tensor(
                                Ht[:], G_ps[:], Wp[:, lo:hi, :], op=ALU.mult
                            )
                        nc.vector.reduce_sum(
                            Hred[:, lo:hi, :],
                            Ht[:].rearrange("p t (d o) -> p t d o", d=D, o=O),
                            axis=mybir.AxisListType.X,
                        )
                        # i-sum + broadcast via the block-diag ones matmul
                        bd_ps = ps_bd.tile(
                            [128, n * D], f32, name=f"bd{rep}_{it}_{lo}", tag="bd"
                        )
                        nc.tensor.matmul(
                            bd_ps[:], J[:], Hred[:, lo:hi, :], start=True, stop=True
                        )
                        bd_v = bd_ps[:].rearrange("p (t d) -> p t d", t=n, d=D)
                        if it == 0:
                            # blog starts at 0: first update is g*bd
                            nc.vector.tensor_scalar_mul(
                                blog[:, lo:hi, :], bd_v, g[:, 0:1]
                            )
                        else:
                            nc.vector.scalar_tensor_tensor(
                                blog[:, lo:hi, :], bd_v, g[:, 0:1],
                                blog[:, lo:hi, :], op0=ALU.mult, op1=ALU.add,
                            )

    _split_excess_waits(nc, 1)
    return nc


_NC_CACHE = {}


def _get_nc(reps=1):
    key = (reps,)
    if key not in _NC_CACHE:
        _NC_CACHE[key] = _build_nc(reps=reps)
    return _NC_CACHE[key]


def _prep_core_inputs(u, W, c):
    r0, r1 = c * RL, (c + 1) * RL
    u2 = np.ascontiguousarray(u[:, r0:r1, :]).reshape(B, KRI)
    u_nat = np.ascontiguousarray(u2.reshape(NB, 128, KRI)).astype(ml_dtypes.bfloat16)
    uT = np.ascontiguousarray(
        np.ascontiguousarray(u2.T).reshape(NT, 128, B).transpose(1, 0, 2)
    ).astype(ml_dtypes.bfloat16)
    Wp2 = np.ascontiguousarray(W[0, r0:r1].transpose(0, 3, 1, 2)).reshape(KRI, DO)
    Wp = np.ascontiguousarray(
        Wp2.reshape(NT, 128, DO).transpose(1, 0, 2)
    ).astype(ml_dtypes.bfloat16)
    return {"u_nat": u_nat, "uT": uT, "Wp": Wp}


def kernel(u, W, _trace=False, _reps=1):
    u = np.asarray(u, dtype=np.float32)
    W = np.asarray(W, dtype=np.float32)
    assert u.shape == (B, R, I_CH) and W.shape == (1, R, D, O, I_CH)
    Jm = np.kron(np.eye(16, dtype=np.float32), np.ones((8, 8), np.float32))
    in_maps = []
    for c in range(N_CORES):
        m = _prep_core_inputs(u, W, c)
        m["Jm"] = Jm
        in_maps.append(m)
    nc = _get_nc(_reps)
    res = run_bass_kernel_spmd(
        nc, in_maps, core_ids=list(range(N_CORES)), trace=_trace
    )
    v = res.results[0]["v_out"].reshape(B, D, O).astype(np.float32)
    if _trace:
        return v, res
    return v
